# revision 18
# baseline (speedup 1.0000x reference)
"""Trainium2 Bass kernel for quantized multi-head attention (ViT-shape).

Computation (per reference):
  q/k/v = x @ W{q,k,v}.T ; per-head scores = (q k^T) * D^-0.5 ;
  fake_quant_per_head(scores) ; softmax ; out = attn @ v ;
  fake_quant_per_head(out) ; merge heads ; out @ Wo.T + bo.

Sharding: data-parallel over batch, 8 images per core on 8 NeuronCores.

Key device-side design (per core, 8 images = 1576 tokens, 4 chunks of 2):
  - All weights host-transposed to [d_in, d_out]; alpha/s_attn folded into
    Wq; 1/s_out folded into Wv (so PV output is pre-divided by s_out);
    s_out folded into Wo; quant zero-offset folded into the output bias.
  - q,k feature-major [128, 6*396] (per-o blocks of 396 = 2 padded images
    of 198) so every bf16 slice lands on a 4-byte boundary.
  - Scores transposed ST[j, i] per head; both j-tiles go into ONE psum
    tile [128, 396] (pieces at cols 0 / 198). ONE tensor_scalar does the
    fake-quant clip + int16 truncate for both tiles, ONE activation does
    exp for both (2-piece access pattern).
  - v stored token-major in 66-wide per-head blocks; col 64 = 1.0 so the
    PV matmul's 65th output column is the softmax denominator (v carries
    1/s_out from the host fold).
  - out-quant merged across heads: reciprocal [il, 12], then
    TT(mult, broadcast inv) x2 banks + TT(min, hib) + TT(max, lob)->int16
    (truncate) where hib/lob are per-column bound tiles built on-device.
  - int16 -> attn_dt copy on GpSimd; PE transpose to feature-major
    (padded psum layout), output projection + bias, 2-piece DMA out.
"""

import os
import numpy as np

B, N, D, H = 64, 197, 768, 12
DH = D // H  # 64
NCORES = 8
BPC = B // NCORES          # 8 images per core
T = BPC * N                # 1576 tokens per core
IMGS_PER_CHUNK = 2
NCHUNK = BPC // IMGS_PER_CHUNK  # 4
TC = IMGS_PER_CHUNK * N    # 394 tokens per chunk
NP = N + 1                 # 198: padded per-image token stride
TCP = IMGS_PER_CHUNK * NP  # 396: padded chunk width
KT = D // 128              # 6 d-tiles
OT = D // 128              # 6 o-tiles
VB = 66                    # per-head v block width (64 data + ones + pad)
Q_LEVELS = 255

_RUNNER_CACHE = {}


def _head_off(h):
    # per-image wide PV psum [128, 1024] (2 banks): heads 0-6 in bank 0,
    # heads 7-11 in bank 1 (a 66-wide block may not cross a 512-f32 bank).
    return VB * h if h < 7 else 512 + VB * (h - 7)


def _build_program(hi_s_attn, lo_s_attn, s_attn, hi_s_out, lo_s_out, s_out,
                   variant, reps=1):
    import concourse.bass as bass
    import concourse.bacc as bacc
    import concourse.mybir as mybir
    from concourse.tile import TileContext

    f32 = mybir.dt.float32
    f32r = mybir.dt.float32r
    bf16 = mybir.dt.bfloat16
    i16 = mybir.dt.int16

    # w_dt: dtype of DMA-loaded projection operands (wq/wk/wv, xT).
    # attn_dt: dtype of on-device-written matmul operands (q/k/E/v/Oq/OT)
    # and of wo (wo must match OT for the output projection).
    if variant == "f32":
        w_dt, attn_dt = f32, f32
    elif variant == "f32r":
        w_dt, attn_dt = f32r, f32
    elif variant == "bf16":
        w_dt, attn_dt = f32r, bf16
    else:
        raise ValueError(variant)
    wo_dt = attn_dt

    nc = bacc.Bacc("TRN2", target_bir_lowering=False, debug=False)

    xT_d = nc.dram_tensor("xT", [D, T], w_dt, kind="ExternalInput").ap()
    wq_d = nc.dram_tensor("wqts", [D, D], w_dt, kind="ExternalInput").ap()
    wk_d = nc.dram_tensor("wkt", [D, D], w_dt, kind="ExternalInput").ap()
    wv_d = nc.dram_tensor("wvts", [D, D], w_dt, kind="ExternalInput").ap()
    wo_d = nc.dram_tensor("wots", [D, D], wo_dt, kind="ExternalInput").ap()
    bo_d = nc.dram_tensor("bof", [D], f32, kind="ExternalInput").ap()
    id_d = nc.dram_tensor("ident", [128, 128], attn_dt, kind="ExternalInput").ap()
    out_d = nc.dram_tensor("outT", [D, T], f32, kind="ExternalOutput").ap()

    Exp = mybir.ActivationFunctionType.Exp
    Ident = mybir.ActivationFunctionType.Identity
    A = mybir.AluOpType

    with TileContext(nc) as tc:
        with (
            tc.tile_pool(name="const", bufs=1) as cpool,
            tc.tile_pool(name="sb", bufs=2) as sb,
            tc.tile_pool(name="ps", bufs=2, space="PSUM") as ps,
        ):
            # chunk 0's activations load ahead of the big weight DMAs so the
            # first projection matmuls aren't stuck behind 9.4MB of weights
            xc0 = sb.tile([128, KT * TC], w_dt, name="xc0", tag="xc", bufs=4)
            for k in range(KT):
                nc.sync.dma_start(
                    out=xc0[:, TC * k:TC * (k + 1)],
                    in_=xT_d[128 * k:128 * (k + 1), 0:TC],
                )
            # ---- resident constants (in first-use order: wq, wk, wv, wo) ----
            wq_sb, wk_sb, wv_sb, wo_sb = [], [], [], []
            for (lst, dram, pfx, dt_) in ((wq_sb, wq_d, "wq", w_dt),
                                          (wk_sb, wk_d, "wk", w_dt),
                                          (wv_sb, wv_d, "wv", w_dt),
                                          (wo_sb, wo_d, "wo", wo_dt)):
                for k in range(KT):
                    t_ = cpool.tile([128, D], dt_, name=f"{pfx}{k}")
                    nc.sync.dma_start(
                        out=t_, in_=dram[128 * k:128 * (k + 1), :])
                    lst.append(t_)
            bo_sb = cpool.tile([128, OT], f32, name="bo")
            for k in range(OT):
                nc.sync.dma_start(
                    out=bo_sb[:, k:k + 1],
                    in_=bo_d[128 * k:128 * (k + 1)].rearrange("(p o) -> p o", o=1),
                )
            ident = cpool.tile([128, 128], attn_dt, name="ident")
            nc.sync.dma_start(out=ident, in_=id_d)
            # per-column out-quant bounds [128, D]: col 64h..64h+64 = head h
            hib = cpool.tile([128, D], f32, name="hib")
            lob = cpool.tile([128, D], f32, name="lob")
            for h in range(H):
                nc.gpsimd.memset(hib[:, DH * h:DH * (h + 1)], float(hi_s_out[h]))
                nc.gpsimd.memset(lob[:, DH * h:DH * (h + 1)], float(lo_s_out[h]))

            import contextlib
            rep_ctx = tc.For_i(0, reps, 1) if reps > 1 else contextlib.nullcontext()
            with rep_ctx:
                _emit_body(nc, tc, sb, ps, locals())
    nc.compile()
    return nc


def _emit_body(nc, tc, sb, ps, env):
    (xT_d, out_d, wq_sb, wk_sb, wv_sb, wo_sb, bo_sb, ident, hib, lob) = (
        env["xT_d"], env["out_d"], env["wq_sb"], env["wk_sb"], env["wv_sb"],
        env["wo_sb"], env["bo_sb"], env["ident"], env["hib"], env["lob"])
    (w_dt, attn_dt, f32, i16) = env["w_dt"], env["attn_dt"], env["f32"], env["i16"]
    (hi_s_attn, lo_s_attn, s_attn) = env["hi_s_attn"], env["lo_s_attn"], env["s_attn"]
    Exp, Ident, A = env["Exp"], env["Ident"], env["A"]

    # transpose of image im is emitted as soon as the NEXT image's attention
    # is queued; the output projection of chunk c-1 is emitted during chunk c
    # (after its projection matmuls) so the PE's FIFO queue never head-of-line
    # blocks on chunk c-1's out-quant chain finishing on DVE/Pool.
    pending = {}

    def emit_tpose(c, im, oqf_pair, otc):
        # per-image transpose: tp pieces it0@0 (128 cols), it1@128 (69)
        for k in range(KT):
            tp = ps.tile([128, 200], attn_dt, name=f"tp{c}{im}{k}", tag="st")
            for it in range(2):
                il = 128 if it == 0 else N - 128
                nc.tensor.transpose(
                    tp[:, 128 * it:128 * it + il],
                    oqf_pair[it][:il, 128 * k:128 * (k + 1)],
                    ident[:il, :il],
                )
            nc.scalar.activation(
                otc[:, TCP * k + NP * im:TCP * k + NP * im + N],
                tp[:, 0:N],
                Ident,
            )

    def emit_outproj(c, otc):
        for o in range(OT):
            op_ = ps.tile([128, TCP], f32, name=f"op{c}{o}", tag="proj")
            for k in range(KT):
                nc.tensor.matmul(
                    op_,
                    lhsT=wo_sb[k][:, 128 * o:128 * (o + 1)],
                    rhs=otc[:, TCP * k:TCP * (k + 1)],
                    start=(k == 0), stop=(k == KT - 1),
                )
            osb = sb.tile([128, TCP], f32, name=f"osb{c}{o}", tag="osb",
                          bufs=3)
            nc.scalar.activation(osb, op_, Ident, bias=bo_sb[:, o:o + 1])
            # store via the ACT queue: FIFO order makes the osb dependency
            # free, and it keeps stores off the SP queue so xT prefetches
            # never head-of-line block behind them
            nc.scalar.dma_start(
                out=out_d[128 * o:128 * (o + 1),
                          TC * c:TC * c + TC].rearrange(
                    "p (i c) -> p i c", c=N),
                in_=osb.rearrange("p (i c) -> p i c", c=NP)[:, :, 0:N],
            )

    for c in range(NCHUNK):
        c0 = TC * c
        # ---- load xT chunk ----
        if c == 0 and env.get("xc0") is not None:
            xc = env["xc0"]
        else:
            xc = sb.tile([128, KT * TC], w_dt, name=f"xc{c}", tag="xc",
                         bufs=4)
            for k in range(KT):
                nc.sync.dma_start(
                    out=xc[:, TC * k:TC * (k + 1)],
                    in_=xT_d[128 * k:128 * (k + 1), c0:c0 + TC],
                )

        # ---- q/k projections (feature-major, padded per-image pieces) ----
        qc = sb.tile([128, OT * TCP], attn_dt, name=f"qc{c}", tag="qc")
        kc = sb.tile([128, OT * TCP], attn_dt, name=f"kc{c}", tag="kc")
        for (wsb, dst) in ((wq_sb, qc), (wk_sb, kc)):
            for o in range(OT):
                pj = ps.tile([128, TC], f32, name=f"pj{c}{o}", tag="proj")
                for k in range(KT):
                    nc.tensor.matmul(
                        pj,
                        lhsT=wsb[k][:, 128 * o:128 * (o + 1)],
                        rhs=xc[:, TC * k:TC * (k + 1)],
                        start=(k == 0), stop=(k == KT - 1),
                    )
                nc.scalar.activation(
                    dst[:, TCP * o:TCP * o + TCP].rearrange(
                        "p (i c) -> p i c", c=NP)[:, :, 0:N],
                    pj.rearrange("p (i c) -> p i c", c=N),
                    Ident,
                )

        # ---- v projection (token-major, per-head 66-col blocks) ----
        # key tiles split 98/99 (PV contraction); scores overlap key 98
        vaug = []
        for im in range(IMGS_PER_CHUNK):
            for tt, (t0, tl) in enumerate(((0, 98), (98, 99))):
                va = sb.tile([128, H * VB], attn_dt,
                             name=f"va{c}{im}{tt}", tag="vaug", bufs=4)
                vav = va.rearrange("p (h c) -> p h c", c=VB)
                for oc in range(2):
                    vp = ps.tile([128, 384], f32,
                                 name=f"vp{c}{im}{tt}{oc}", tag="proj")
                    for k in range(KT):
                        nc.tensor.matmul(
                            vp[:tl],
                            lhsT=xc[:, TC * k + N * im + t0:
                                    TC * k + N * im + t0 + tl],
                            rhs=wv_sb[k][:, 384 * oc:384 * (oc + 1)],
                            start=(k == 0), stop=(k == KT - 1),
                        )
                    nc.vector.tensor_copy(
                        vav[:tl, 6 * oc:6 * (oc + 1), 0:64],
                        vp[:tl].rearrange("p (h c) -> p h c", c=64),
                    )
                # ones columns (64: denominator source, 65: keep finite)
                nc.gpsimd.memset(vav[:tl, :, 64:66], 1.0)
                vaug.append(va)

        # previous chunk's transpose + output projection
        if deferred[0] is not None:
            emit_back(*deferred[0])
            deferred[0] = None

        # ---- attention per image ----
        oqf_all = []
        for im in range(IMGS_PER_CHUNK):
            pv = []
            for it in range(2):
                pvt = ps.tile([128, 1024], f32,
                              name=f"pv{c}{im}{it}", tag="pv")
                pv.append(pvt)
            for h in range(H):
                o, row = h // 2, (h % 2) * 64
                base = TCP * o + NP * im
                # scores^T: both key tiles (0-98, 98-196) into one psum
                # tile (pieces at cols 0/198), each 99 rows from base 0
                sp = ps.tile([128, TCP], f32, name=f"sp{c}{im}{h}", tag="st")
                for jt, k0 in enumerate((0, 98)):
                    nc.tensor.matmul(
                        sp[:99, NP * jt:NP * jt + N],
                        lhsT=kc[row:row + 64, base + k0:base + k0 + 99],
                        rhs=qc[row:row + 64, base:base + N],
                        start=True, stop=True,
                    )
                spv = sp.rearrange("p (j c) -> p j c", c=NP)[:99, :, 0:N]
                q16 = sb.tile([128, TCP], i16, name=f"q16{c}{im}{h}",
                              tag="q16", bufs=4)
                q16v = q16.rearrange("p (j c) -> p j c", c=NP)[:99, :, 0:N]
                nc.vector.tensor_scalar(
                    out=q16v, in0=spv,
                    scalar1=float(hi_s_attn[h]),
                    scalar2=float(lo_s_attn[h]),
                    op0=A.min, op1=A.max,
                )
                ef = sb.tile([128, TCP], attn_dt, name=f"ef{c}{im}{h}",
                             tag="ef", bufs=4)
                efv = ef.rearrange("p (j c) -> p j c", c=NP)[:99, :, 0:N]
                nc.scalar.activation(efv, q16v, Exp, scale=float(s_attn[h]))
                off = _head_off(h)
                for it in range(2):
                    il = 128 if it == 0 else N - 128
                    for jt, jl in enumerate((98, 99)):
                        nc.tensor.matmul(
                            pv[it][:il, off:off + 65],
                            lhsT=ef[:jl, NP * jt + 128 * it:
                                    NP * jt + 128 * it + il],
                            rhs=vaug[2 * im + jt].rearrange(
                                "p (h c) -> p h c", c=VB)[:jl, h, 0:65],
                            start=(jt == 0), stop=(jt == 1),
                        )
            # normalization + out-quant (heads merged)
            for it in range(2):
                il = 128 if it == 0 else N - 128
                b0 = pv[it][:il, 0:7 * VB].rearrange("p (h c) -> p h c", c=VB)
                b1 = pv[it][:il, 512:512 + 5 * VB].rearrange(
                    "p (h c) -> p h c", c=VB)
                inv = sb.tile([128, H], f32, name=f"inv{c}{im}{it}",
                              tag="inv", bufs=4)
                nc.vector.reciprocal(inv[:il, 0:7], b0[:, :, 64])
                nc.vector.reciprocal(inv[:il, 7:12], b1[:, :, 64])
                oqt = sb.tile([128, D], f32, name=f"oqt{c}{im}{it}", tag="oqt")
                nc.vector.tensor_tensor(
                    out=oqt[:il, 0:448].rearrange("p (h c) -> p h c", c=64),
                    in0=b0[:, :, 0:64],
                    in1=inv[:il, 0:7].unsqueeze(2).broadcast_to((il, 7, 64)),
                    op=A.mult,
                )
                nc.vector.tensor_tensor(
                    out=oqt[:il, 448:768].rearrange("p (h c) -> p h c", c=64),
                    in0=b1[:, :, 0:64],
                    in1=inv[:il, 7:12].unsqueeze(2).broadcast_to((il, 5, 64)),
                    op=A.mult,
                )
                oqm = sb.tile([128, D], f32, name=f"oqm{c}{im}{it}", tag="oqm")
                nc.vector.tensor_tensor(
                    out=oqm[:il], in0=oqt[:il], in1=hib[:il], op=A.min)
                oqi = sb.tile([128, D], i16, name=f"oqi{c}{im}{it}", tag="oqi")
                nc.vector.tensor_tensor(
                    out=oqi[:il], in0=oqm[:il], in1=lob[:il], op=A.max)
                oqf = sb.tile([128, D], attn_dt, name=f"oqf{c}{im}{it}",
                              tag="oqf", bufs=8)
                nc.gpsimd.tensor_copy(oqf[:il], oqi[:il])
                oqf_all.append(oqf)

        deferred[0] = (c, oqf_all)

    emit_back(*deferred[0])


def _prepare_host_inputs(x, Wq, Wk, Wv, Wo, bo,
                         qmin_attn, qmax_attn, qmin_out, qmax_out, variant):
    """Returns (in_maps list per core, qparam tuple)."""
    f = np.float32
    alpha = np.float32(D ** -0.5)
    s_attn = ((qmax_attn - qmin_attn) / Q_LEVELS).astype(f)
    s_out = ((qmax_out - qmin_out) / Q_LEVELS).astype(f)
    hi_s_attn = (qmax_attn / s_attn).astype(f)
    lo_s_attn = (qmin_attn / s_attn).astype(f)
    hi_s_out = (qmax_out / s_out).astype(f)
    lo_s_out = (qmin_out / s_out).astype(f)

    head_of_o = np.arange(D) // DH
    wqts = np.ascontiguousarray(
        (Wq * (alpha / s_attn[head_of_o])[:, None]).T).astype(f)
    wkt = np.ascontiguousarray(Wk.T).astype(f)
    # fold 1/s_out into v so PV output is pre-divided by s_out
    wvts = np.ascontiguousarray(
        (Wv / s_out[head_of_o][:, None]).T).astype(f)
    wots = np.ascontiguousarray((Wo * s_out[head_of_o][None, :]).T).astype(f)
    bof = (bo + Wo @ qmin_out[head_of_o]).astype(f)

    if variant == "bf16":
        import ml_dtypes
        adt = ml_dtypes.bfloat16
        wots_c, ident = wots.astype(adt), np.eye(128, dtype=adt)
    else:
        wots_c, ident = wots, np.eye(128, dtype=f)

    in_maps = []
    for i in range(NCORES):
        xs = np.ascontiguousarray(
            x[BPC * i:BPC * (i + 1)].reshape(T, D).T).astype(f)
        in_maps.append(dict(xT=xs, wqts=wqts, wkt=wkt, wvts=wvts, wots=wots_c,
                            bof=bof, ident=ident))
    qparams = (hi_s_attn, lo_s_attn, s_attn, hi_s_out, lo_s_out, s_out)
    return in_maps, qparams


class _Runner:
    """Compiled SPMD executable over 8 cores (PJRT path, jit cached)."""

    def __init__(self, nc):
        import jax
        import concourse.mybir as mybir
        from concourse import bass2jax
        from jax.sharding import Mesh, PartitionSpec
        from jax.experimental.shard_map import shard_map

        bass2jax.install_neuronx_cc_hook()
        self.nc = nc
        assert nc.dbg_addr is None
        partition_name = (nc.partition_id_tensor.name
                          if nc.partition_id_tensor else None)

        in_names, out_names, out_avals, zero_outs = [], [], [], []
        for alloc in nc.m.functions[0].allocations:
            if not isinstance(alloc, mybir.MemoryLocationSet):
                continue
            name = alloc.memorylocations[0].name
            if alloc.kind == "ExternalInput":
                if name != partition_name:
                    in_names.append(name)
            elif alloc.kind == "ExternalOutput":
                shape = tuple(alloc.tensor_shape)
                dtype = mybir.dt.np(alloc.dtype)
                out_names.append(name)
                out_avals.append(jax.core.ShapedArray(shape, dtype))
                zero_outs.append(np.zeros(shape, dtype))
        self.in_names, self.out_names = in_names, out_names
        self.out_avals, self.zero_outs = out_avals, zero_outs
        n_params, n_outs = len(in_names), len(out_avals)
        all_names = list(in_names) + list(out_names)
        if partition_name is not None:
            all_names.append(partition_name)
        all_names = tuple(all_names)

        def _body(*args):
            operands = list(args)
            if partition_name is not None:
                operands.append(bass2jax.partition_id_tensor())
            outs = bass2jax._bass_exec_p.bind(
                *operands,
                out_avals=tuple(out_avals),
                in_names=all_names,
                out_names=tuple(out_names),
                lowering_input_output_aliases=(),
                sim_require_finite=True,
                sim_require_nnan=True,
                nc=nc,
            )
            return tuple(outs)

        devices = jax.devices()[:NCORES]
        mesh = Mesh(np.asarray(devices), ("core",))
        self.mesh = mesh
        self.spec = PartitionSpec("core")
        self.sharded = jax.jit(
            shard_map(_body, mesh=mesh,
                      in_specs=(PartitionSpec("core"),) * (n_params + n_outs),
                      out_specs=(PartitionSpec("core"),) * n_outs,
                      check_rep=False),
            donate_argnums=tuple(range(n_params, n_params + n_outs)),
            keep_unused=True,
        )
        import jax.numpy as jnp
        from jax.sharding import NamedSharding
        zshardings = tuple(NamedSharding(mesh, self.spec) for _ in zero_outs)
        zshapes = [(NCORES * z.shape[0], *z.shape[1:]) for z in zero_outs]
        zdtypes = [z.dtype for z in zero_outs]
        self.zeros_fn = jax.jit(
            lambda: tuple(jnp.zeros(s, d) for s, d in zip(zshapes, zdtypes)),
            out_shardings=zshardings,
        )

    def device_put_inputs(self, concat_in):
        import jax
        from jax.sharding import NamedSharding
        sh = NamedSharding(self.mesh, self.spec)
        return [jax.device_put(a, sh) for a in concat_in]

    def concat_inputs(self, in_maps):
        return [np.concatenate([np.asarray(m[name]) for m in in_maps], axis=0)
                for name in self.in_names]

    def run_raw(self, concat_in):
        return self.sharded(*concat_in, *self.zeros_fn())

    def run(self, in_maps):
        out_arrs = self.run_raw(self.concat_inputs(in_maps))
        return [
            {name: np.asarray(out_arrs[i]).reshape(
                NCORES, *self.out_avals[i].shape)[c]
             for i, name in enumerate(self.out_names)}
            for c in range(NCORES)
        ]


def get_runner(qparams, variant):
    key = (variant,) + tuple(p.tobytes() for p in qparams)
    if key not in _RUNNER_CACHE:
        _RUNNER_CACHE[key] = _Runner(_build_program(*qparams, variant))
    return _RUNNER_CACHE[key]


def kernel(x, Wq, Wk, Wv, Wo, bo, qmin_attn, qmax_attn, qmin_out, qmax_out):
    variant = os.environ.get("KVAR", "f32")
    in_maps, qparams = _prepare_host_inputs(
        np.asarray(x, np.float32), np.asarray(Wq, np.float32),
        np.asarray(Wk, np.float32), np.asarray(Wv, np.float32),
        np.asarray(Wo, np.float32), np.asarray(bo, np.float32),
        np.asarray(qmin_attn, np.float32), np.asarray(qmax_attn, np.float32),
        np.asarray(qmin_out, np.float32), np.asarray(qmax_out, np.float32),
        variant,
    )
    runner = get_runner(qparams, variant)
    results = runner.run(in_maps)
    out = np.empty((B, N, D), np.float32)
    for i in range(NCORES):
        out[BPC * i:BPC * (i + 1)] = results[i]["outT"].T.reshape(BPC, N, D)
    kernel.last_runner = runner
    kernel.last_in_maps = in_maps
    return out
